# revision 22
# baseline (speedup 1.0000x reference)
"""Trainium2 Bass kernel for nn_Differ_Amplifier (gnn_message_passing).

Reference computation (per layer i, h0 = x [N, H]):
    represent = (N*h - colsum(h)) / (N-1)
    h = represent @ W_i.T + h
    out_i = sigmoid(h @ W_ff.T + b_ff)

Reformulation (exact algebra, validated vs fp64):
  - With V_i = I + c*W_i^T, c = N/(N-1): h_{i+1} = h_i @ V_i - 1*b_i
    (rank-1 bias row), and colsum(h) is invariant across layers.
  - Composing on the host: M_{i+1} = V_0 @ ... @ V_i,
    G_i = M_{i+1} @ W_ff^T, c_i = b_ff + (total/N) @ (W_ff^T - G_i)
    gives out_i = sigmoid(x @ G_i + c_i).
  - `kernel()` receives the FULL inputs, so total = colsum(x), every G_i,
    every bias row c_i, AND the transposed fp16 x^T are all computed on
    the host. The device does no collectives, no transposes, no bias
    math: just matmuls + bias-add + sigmoid + streaming output DMA.

Device schedule per core (rows = 4096, sharded on N across 8 cores):
  - x^T arrives pre-transposed/fp16 as [128, RG, KC, 512]
    (hidden-chunk on partitions, rows in free dim), 4 KB/partition runs.
  - Per 128-row tile: 16 matmuls (k-chunk outer for weight reuse,
    layer inner) accumulate all 4 layers into ONE [128, 2048] PSUM
    tile (4 banks, one 512-slice per layer); a single [128, 2048]
    Vector add applies all 4 bias rows (broadcast-DMA'd from DRAM at
    t=0); a single [128, 2048] ACT sigmoid evicts to SBUF; one 1 MB
    DMA writes all 4 layers for the tile.
  - Output DMA starts after the first row tile (~7 us), so the 32 MB
    output write overlaps the whole compute instead of trailing it.
"""

import numpy as np

import concourse.bass as bass
import concourse.tile as tile
from concourse import bacc, mybir
from concourse import bass_utils

N_CORES = 8
N_TOTAL = 32768
H = 512
L = 4
P = 128
KC = H // P  # 4 k-chunks of the hidden dim
F16 = mybir.dt.float16
F32 = mybir.dt.float32
SIG = mybir.ActivationFunctionType.Sigmoid

TRACE = False


def build(rows=N_TOTAL // N_CORES):
    """Build the SPMD kernel for one core owning `rows` rows."""
    nc = bacc.Bacc(
        "TRN2", target_bir_lowering=False, debug=False, num_devices=N_CORES
    )
    assert rows % 1024 == 0
    G = rows // 1024  # output groups: 8 row tiles whose output DMAs batch
    xt = nc.dram_tensor("xt", [P, G, 2, KC, 512], F16,
                        kind="ExternalInput").ap()
    gft = nc.dram_tensor("gft", [L, P, KC, H], F16, kind="ExternalInput").ap()
    cvec = nc.dram_tensor("cvec", [1, L * H], F32, kind="ExternalInput").ap()
    out = nc.dram_tensor("out", [L, rows, H], F32, kind="ExternalOutput").ap()
    # Row r = g*1024 + p*8 + u: partition p owns 8 CONSECUTIVE rows of each
    # group, so a grouped output DMA moves 16 KB contiguous DRAM runs per
    # partition per layer. DMA throughput is packet-rate-bound (~55 ns
    # fixed + ~43 ns/KB per per-partition run on each of 16 engines), so
    # big runs are what buys write bandwidth.
    out_r = out.rearrange("l (g p u) d -> p l g u d", p=P, u=8)

    with tile.TileContext(nc) as tc:
        with (
            tc.tile_pool(name="wpool", bufs=1) as wpool,
            tc.tile_pool(name="xpool", bufs=1) as xpool,
            tc.tile_pool(name="opool", bufs=6) as opool,
            tc.tile_pool(name="psum", bufs=1, space="PSUM") as psum,
        ):
            # ---- input DMAs ----------------------------------------------
            # gft[0] + first x pair first so matmuls start ASAP; everything
            # on the sync ring (gpsimd's TileContext DRAIN is ~17 us and
            # would delay the first matmul). x pairs = 8 KB read packets.
            # x chunks: one per half-group (0.5 MB, 4 KB read packets).
            # gft[0] + the first group's two halves go first so matmuls
            # start ASAP.
            xts = [
                [
                    xpool.tile([P, KC, 512], F16, tag=f"x{g}_{hf}",
                               name=f"x{g}_{hf}")
                    for hf in range(2)
                ]
                for g in range(G)
            ]
            gft_sb = [
                wpool.tile([P, KC, H], F16, tag=f"gf{i}", name=f"gf{i}")
                for i in range(L)
            ]
            nc.sync.dma_start(out=gft_sb[0], in_=gft[0])
            nc.sync.dma_start(out=xts[0][0], in_=xt[:, 0, 0])
            nc.sync.dma_start(out=xts[0][1], in_=xt[:, 0, 1])
            for i in range(1, L):
                nc.sync.dma_start(out=gft_sb[i], in_=gft[i])
            for g in range(1, G):
                for hf in range(2):
                    nc.sync.dma_start(out=xts[g][hf], in_=xt[:, g, hf])
            cb = wpool.tile([P, L * H], F32, tag="cb")
            c_bcast = bass.AP(
                tensor=cvec.tensor,
                offset=cvec.offset,
                ap=[[0, P]] + list(cvec.ap[1:]),
            )
            nc.scalar.dma_start(out=cb, in_=c_bcast)

            # ---- main loop ------------------------------------------------
            # Per 128-row tile: 16 matmuls accumulate all 4 layers into one
            # [128, 2048] PSUM tile (layer-outer order so each 512-slice
            # finishes early and its bias add overlaps remaining matmuls),
            # then one [128, 2048] sigmoid into the group output buffer.
            # One 4 MB DMA per group of 4 tiles streams the result out.
            # One [128, 512] PSUM tile (= one bank) per (tile, layer) slice,
            # rotating over all 8 banks. PSUM hazards are tile-granular, so
            # a big multi-bank tile would chain each new accumulation to
            # sigmoids of OTHER slices in the same tile (~1.34 us/slice
            # critical cycle); per-bank tiles put the write-after-read
            # partner 8 slices back and Tensor runs free.
            # Layer-outer within each group: the (group, layer) output block
            # [128 rows x 8 tiles x 512] completes every ~7 us, so 2 MB
            # output DMAs (16 KB runs) issue at a smooth cadence. The very
            # last block splits in two so its first half drains while the
            # second half computes.
            for g in range(G):
                for i in range(L):
                    sl = slice(i * H, (i + 1) * H)
                    ob = opool.tile([P, 8, H], F32, tag="ob",
                                    name=f"ob{g}_{i}")
                    last = (g == G - 1 and i == L - 1)
                    for u in range(8):
                        cs = slice((u % 4) * P, (u % 4 + 1) * P)
                        pf = psum.tile([P, H], F32, tag="z", bufs=8,
                                       name=f"z{g}_{u}_{i}")
                        for k in range(KC):
                            nc.tensor.matmul(
                                pf,
                                xts[g][u // 4][:, k, cs],
                                gft_sb[i][:, k, :],
                                start=(k == 0),
                                stop=(k == KC - 1),
                            )
                        nc.vector.tensor_add(pf, pf, cb[:, sl])
                        nc.scalar.activation(ob[:, u, :], pf, SIG)
                        if last and u == 3:
                            nc.sync.dma_start(
                                out=out_r[:, i, g, 0:4, :], in_=ob[:, 0:4, :]
                            )
                    if last:
                        nc.sync.dma_start(
                            out=out_r[:, i, g, 4:8, :], in_=ob[:, 4:8, :]
                        )
                    else:
                        nc.sync.dma_start(out=out_r[:, i, g, :, :], in_=ob)

    nc.compile()
    return nc


def _prep_weights(x_full, Ws, W_ff, b_ff):
    """Host-side: compose G_i, bias rows c_i (fp64), pack for the device."""
    n = x_full.shape[0]
    c = n / (n - 1.0)
    eye = np.eye(H, dtype=np.float64)
    wfT = W_ff.astype(np.float64).T  # [H, OUT]
    total = x_full.sum(axis=0, dtype=np.float64)  # [H]
    # device layout [L, P, KC, H]: partition p, chunk k holds G[k*P+p, :]
    gf = np.empty((L, P, KC, H), dtype=np.float16)
    cv = np.empty((1, L * H), dtype=np.float32)
    M = eye.copy()
    for i in range(L):
        M = M @ (eye + c * Ws[i].astype(np.float64).T)  # M_{i+1}
        Gi = M @ wfT
        gf[i] = Gi.astype(np.float16).reshape(KC, P, H).transpose(1, 0, 2)
        cv[0, i * H:(i + 1) * H] = (
            b_ff.astype(np.float64) + (total / n) @ (wfT - Gi)
        ).astype(np.float32)
    return gf, cv


def _prep_x(x_core):
    """[rows, H] fp32 -> [P, G, 2, KC, 512] fp16.

    Hidden dim on partitions (k*128+p_h); free column uu*128+p_out of
    half hf of group g maps to row g*1024 + p_out*8 + hf*4 + uu, so
    output partitions own 8 consecutive rows (16 KB contiguous output
    DMA runs) while x still loads in 0.5 MB half-group chunks.
    """
    rows = x_core.shape[0]
    g = rows // 1024
    return (
        x_core.reshape(g, P, 2, 4, KC, P)   # [g, p_out, hf, uu, k, p_h]
        .transpose(5, 0, 2, 4, 3, 1)        # [p_h, g, hf, k, uu, p_out]
        .astype(np.float16, order="C")
        .reshape(P, g, 2, KC, 512)
    )


_CACHE = {}


def kernel(input, Ws, W_ff, b_ff):
    x = np.asarray(input, dtype=np.float32)[0]  # [N, H]
    Ws = np.asarray(Ws, dtype=np.float32)
    W_ff = np.asarray(W_ff, dtype=np.float32)
    b_ff = np.asarray(b_ff, dtype=np.float32)
    n, h = x.shape
    rows = n // N_CORES

    if "nc" not in _CACHE:
        _CACHE["nc"] = build(rows=rows)
    nc = _CACHE["nc"]

    gf, cv = _prep_weights(x, Ws, W_ff, b_ff)
    in_maps = [
        {
            "xt": _prep_x(x[c * rows:(c + 1) * rows]),
            "gft": gf,
            "cvec": cv,
        }
        for c in range(N_CORES)
    ]
    res = bass_utils.run_bass_kernel_spmd(
        nc, in_maps, core_ids=list(range(N_CORES)), trace=TRACE
    )
    _CACHE["last_res"] = res
    out = np.concatenate([res.results[c]["out"] for c in range(N_CORES)], axis=1)
    return out.astype(np.float32)


# revision 23
# speedup vs baseline: 1.0047x; 1.0047x over previous
"""Trainium2 Bass kernel for nn_Differ_Amplifier (gnn_message_passing).

Reference computation (per layer i, h0 = x [N, H]):
    represent = (N*h - colsum(h)) / (N-1)
    h = represent @ W_i.T + h
    out_i = sigmoid(h @ W_ff.T + b_ff)

Reformulation (exact algebra, validated vs fp64):
  - With V_i = I + c*W_i^T, c = N/(N-1): h_{i+1} = h_i @ V_i - 1*b_i
    (rank-1 bias row), and colsum(h) is invariant across layers.
  - Composing on the host: M_{i+1} = V_0 @ ... @ V_i,
    G_i = M_{i+1} @ W_ff^T, c_i = b_ff + (total/N) @ (W_ff^T - G_i)
    gives out_i = sigmoid(x @ G_i + c_i).
  - `kernel()` receives the FULL inputs, so total = colsum(x), every G_i,
    every bias row c_i, AND the transposed fp16 x^T are all computed on
    the host. The device does no collectives, no transposes, no bias
    math: just matmuls + bias-add + sigmoid + streaming output DMA.

Device schedule per core (rows = 4096, sharded on N across 8 cores):
  - x^T arrives pre-transposed/fp16 as [128, RG, KC, 512]
    (hidden-chunk on partitions, rows in free dim), 4 KB/partition runs.
  - Per 128-row tile: 16 matmuls (k-chunk outer for weight reuse,
    layer inner) accumulate all 4 layers into ONE [128, 2048] PSUM
    tile (4 banks, one 512-slice per layer); a single [128, 2048]
    Vector add applies all 4 bias rows (broadcast-DMA'd from DRAM at
    t=0); a single [128, 2048] ACT sigmoid evicts to SBUF; one 1 MB
    DMA writes all 4 layers for the tile.
  - Output DMA starts after the first row tile (~7 us), so the 32 MB
    output write overlaps the whole compute instead of trailing it.
"""

import numpy as np

import concourse.bass as bass
import concourse.tile as tile
from concourse import bacc, mybir
from concourse import bass_utils

N_CORES = 8
N_TOTAL = 32768
H = 512
L = 4
P = 128
KC = H // P  # 4 k-chunks of the hidden dim
F16 = mybir.dt.float16
F32 = mybir.dt.float32
SIG = mybir.ActivationFunctionType.Sigmoid

TRACE = False


def build(rows=N_TOTAL // N_CORES):
    """Build the SPMD kernel for one core owning `rows` rows."""
    nc = bacc.Bacc(
        "TRN2", target_bir_lowering=False, debug=False, num_devices=N_CORES
    )
    assert rows % 1024 == 0
    G = rows // 1024  # output groups: 8 row tiles whose output DMAs batch
    xt = nc.dram_tensor("xt", [P, G, 2, KC, 512], F16,
                        kind="ExternalInput").ap()
    gft = nc.dram_tensor("gft", [L, P, KC, H], F16, kind="ExternalInput").ap()
    cvec = nc.dram_tensor("cvec", [1, L * H], F32, kind="ExternalInput").ap()
    out = nc.dram_tensor("out", [L, rows, H], F32, kind="ExternalOutput").ap()
    # Row r = g*1024 + p*8 + u: partition p owns 8 CONSECUTIVE rows of each
    # group, so a grouped output DMA moves 16 KB contiguous DRAM runs per
    # partition per layer. DMA throughput is packet-rate-bound (~55 ns
    # fixed + ~43 ns/KB per per-partition run on each of 16 engines), so
    # big runs are what buys write bandwidth.
    out_r = out.rearrange("l (g p u) d -> p l g u d", p=P, u=8)

    with tile.TileContext(nc) as tc:
        with (
            tc.tile_pool(name="wpool", bufs=1) as wpool,
            tc.tile_pool(name="xpool", bufs=1) as xpool,
            tc.tile_pool(name="opool", bufs=6) as opool,
            tc.tile_pool(name="psum", bufs=1, space="PSUM") as psum,
        ):
            # ---- input DMAs ----------------------------------------------
            # gft[0] + first x pair first so matmuls start ASAP; everything
            # on the sync ring (gpsimd's TileContext DRAIN is ~17 us and
            # would delay the first matmul). x pairs = 8 KB read packets.
            # x chunks: one per half-group (0.5 MB, 4 KB read packets).
            # gft[0] + the first group's two halves go first so matmuls
            # start ASAP.
            xts = [
                [
                    xpool.tile([P, KC, 512], F16, tag=f"x{g}_{hf}",
                               name=f"x{g}_{hf}")
                    for hf in range(2)
                ]
                for g in range(G)
            ]
            gft_sb = [
                wpool.tile([P, KC, H], F16, tag=f"gf{i}", name=f"gf{i}")
                for i in range(L)
            ]
            nc.sync.dma_start(out=gft_sb[0], in_=gft[0])
            nc.sync.dma_start(out=xts[0][0], in_=xt[:, 0, 0])
            nc.sync.dma_start(out=xts[0][1], in_=xt[:, 0, 1])
            for i in range(1, L):
                nc.sync.dma_start(out=gft_sb[i], in_=gft[i])
            for g in range(1, G):
                for hf in range(2):
                    nc.sync.dma_start(out=xts[g][hf], in_=xt[:, g, hf])
            cb = wpool.tile([P, L * H], F32, tag="cb")
            c_bcast = bass.AP(
                tensor=cvec.tensor,
                offset=cvec.offset,
                ap=[[0, P]] + list(cvec.ap[1:]),
            )
            nc.scalar.dma_start(out=cb, in_=c_bcast)

            # ---- main loop ------------------------------------------------
            # Per 128-row tile: 16 matmuls accumulate all 4 layers into one
            # [128, 2048] PSUM tile (layer-outer order so each 512-slice
            # finishes early and its bias add overlaps remaining matmuls),
            # then one [128, 2048] sigmoid into the group output buffer.
            # One 4 MB DMA per group of 4 tiles streams the result out.
            # One [128, 512] PSUM tile (= one bank) per (tile, layer) slice,
            # rotating over all 8 banks. PSUM hazards are tile-granular, so
            # a big multi-bank tile would chain each new accumulation to
            # sigmoids of OTHER slices in the same tile (~1.34 us/slice
            # critical cycle); per-bank tiles put the write-after-read
            # partner 8 slices back and Tensor runs free.
            # Layer-outer within each group: the (group, layer) output block
            # [128 rows x 8 tiles x 512] completes every ~7 us, so 2 MB
            # output DMAs (16 KB runs) issue at a smooth cadence. The very
            # last block splits in two so its first half drains while the
            # second half computes.
            for g in range(G):
                for i in range(L):
                    sl = slice(i * H, (i + 1) * H)
                    ob = opool.tile([P, 8, H], F32, tag="ob",
                                    name=f"ob{g}_{i}")
                    last = (g == G - 1 and i == L - 1)
                    for u in range(8):
                        cs = slice((u % 4) * P, (u % 4 + 1) * P)
                        pf = psum.tile([P, H], F32, tag="z", bufs=8,
                                       name=f"z{g}_{u}_{i}")
                        for k in range(KC):
                            nc.tensor.matmul(
                                pf,
                                xts[g][u // 4][:, k, cs],
                                gft_sb[i][:, k, :],
                                start=(k == 0),
                                stop=(k == KC - 1),
                            )
                        nc.vector.tensor_add(pf, pf, cb[:, sl])
                        nc.scalar.activation(ob[:, u, :], pf, SIG)
                        if last and u == 3:
                            nc.sync.dma_start(
                                out=out_r[:, i, g, 0:4, :], in_=ob[:, 0:4, :]
                            )
                    # alternate hardware rings: two queues keep twice the
                    # packets outstanding, which sets the drain rate once
                    # compute stops feeding new descriptors
                    q = nc.sync if (g * L + i) % 2 == 0 else nc.scalar
                    if last:
                        nc.sync.dma_start(
                            out=out_r[:, i, g, 4:6, :], in_=ob[:, 4:6, :]
                        )
                        nc.scalar.dma_start(
                            out=out_r[:, i, g, 6:8, :], in_=ob[:, 6:8, :]
                        )
                    else:
                        q.dma_start(out=out_r[:, i, g, :, :], in_=ob)

    nc.compile()
    return nc


def _prep_weights(x_full, Ws, W_ff, b_ff):
    """Host-side: compose G_i, bias rows c_i (fp64), pack for the device."""
    n = x_full.shape[0]
    c = n / (n - 1.0)
    eye = np.eye(H, dtype=np.float64)
    wfT = W_ff.astype(np.float64).T  # [H, OUT]
    total = x_full.sum(axis=0, dtype=np.float64)  # [H]
    # device layout [L, P, KC, H]: partition p, chunk k holds G[k*P+p, :]
    gf = np.empty((L, P, KC, H), dtype=np.float16)
    cv = np.empty((1, L * H), dtype=np.float32)
    M = eye.copy()
    for i in range(L):
        M = M @ (eye + c * Ws[i].astype(np.float64).T)  # M_{i+1}
        Gi = M @ wfT
        gf[i] = Gi.astype(np.float16).reshape(KC, P, H).transpose(1, 0, 2)
        cv[0, i * H:(i + 1) * H] = (
            b_ff.astype(np.float64) + (total / n) @ (wfT - Gi)
        ).astype(np.float32)
    return gf, cv


def _prep_x(x_core):
    """[rows, H] fp32 -> [P, G, 2, KC, 512] fp16.

    Hidden dim on partitions (k*128+p_h); free column uu*128+p_out of
    half hf of group g maps to row g*1024 + p_out*8 + hf*4 + uu, so
    output partitions own 8 consecutive rows (16 KB contiguous output
    DMA runs) while x still loads in 0.5 MB half-group chunks.
    """
    rows = x_core.shape[0]
    g = rows // 1024
    return (
        x_core.reshape(g, P, 2, 4, KC, P)   # [g, p_out, hf, uu, k, p_h]
        .transpose(5, 0, 2, 4, 3, 1)        # [p_h, g, hf, k, uu, p_out]
        .astype(np.float16, order="C")
        .reshape(P, g, 2, KC, 512)
    )


_CACHE = {}


def kernel(input, Ws, W_ff, b_ff):
    x = np.asarray(input, dtype=np.float32)[0]  # [N, H]
    Ws = np.asarray(Ws, dtype=np.float32)
    W_ff = np.asarray(W_ff, dtype=np.float32)
    b_ff = np.asarray(b_ff, dtype=np.float32)
    n, h = x.shape
    rows = n // N_CORES

    if "nc" not in _CACHE:
        _CACHE["nc"] = build(rows=rows)
    nc = _CACHE["nc"]

    gf, cv = _prep_weights(x, Ws, W_ff, b_ff)
    in_maps = [
        {
            "xt": _prep_x(x[c * rows:(c + 1) * rows]),
            "gft": gf,
            "cvec": cv,
        }
        for c in range(N_CORES)
    ]
    res = bass_utils.run_bass_kernel_spmd(
        nc, in_maps, core_ids=list(range(N_CORES)), trace=TRACE
    )
    _CACHE["last_res"] = res
    out = np.concatenate([res.results[c]["out"] for c in range(N_CORES)], axis=1)
    return out.astype(np.float32)


# revision 25
# speedup vs baseline: 1.0125x; 1.0078x over previous
"""Trainium2 Bass kernel for nn_Differ_Amplifier (gnn_message_passing).

Reference computation (per layer i, h0 = x [N, H]):
    represent = (N*h - colsum(h)) / (N-1)
    h = represent @ W_i.T + h
    out_i = sigmoid(h @ W_ff.T + b_ff)

Reformulation (exact algebra, validated vs fp64):
  - With V_i = I + c*W_i^T, c = N/(N-1): h_{i+1} = h_i @ V_i - 1*b_i
    (rank-1 bias row), and colsum(h) is invariant across layers.
  - Composing on the host: M_{i+1} = V_0 @ ... @ V_i,
    G_i = M_{i+1} @ W_ff^T, c_i = b_ff + (total/N) @ (W_ff^T - G_i)
    gives out_i = sigmoid(x @ G_i + c_i).
  - `kernel()` receives the FULL inputs, so total = colsum(x), every G_i,
    every bias row c_i, AND the transposed fp16 x^T are all computed on
    the host. The device does no collectives, no transposes, no bias
    math: just matmuls + bias-add + sigmoid + streaming output DMA.

Device schedule per core (rows = 4096, sharded on N across 8 cores):
  - x^T arrives pre-transposed/fp16 as [128, RG, KC, 512]
    (hidden-chunk on partitions, rows in free dim), 4 KB/partition runs.
  - Per 128-row tile: 16 matmuls (k-chunk outer for weight reuse,
    layer inner) accumulate all 4 layers into ONE [128, 2048] PSUM
    tile (4 banks, one 512-slice per layer); a single [128, 2048]
    Vector add applies all 4 bias rows (broadcast-DMA'd from DRAM at
    t=0); a single [128, 2048] ACT sigmoid evicts to SBUF; one 1 MB
    DMA writes all 4 layers for the tile.
  - Output DMA starts after the first row tile (~7 us), so the 32 MB
    output write overlaps the whole compute instead of trailing it.
"""

import numpy as np

import concourse.bass as bass
import concourse.tile as tile
from concourse import bacc, mybir
from concourse import bass_utils

N_CORES = 8
N_TOTAL = 32768
H = 512
L = 4
P = 128
KC = H // P  # 4 k-chunks of the hidden dim
F16 = mybir.dt.float16
F32 = mybir.dt.float32
SIG = mybir.ActivationFunctionType.Sigmoid

TRACE = False


def build(rows=N_TOTAL // N_CORES):
    """Build the SPMD kernel for one core owning `rows` rows."""
    nc = bacc.Bacc(
        "TRN2", target_bir_lowering=False, debug=False, num_devices=N_CORES
    )
    assert rows % 1024 == 0
    G = rows // 1024  # output groups: 8 row tiles whose output DMAs batch
    xt = nc.dram_tensor("xt", [P, G, 2, KC, 512], F16,
                        kind="ExternalInput").ap()
    gft = nc.dram_tensor("gft", [L, P, KC, H], F16, kind="ExternalInput").ap()
    cvec = nc.dram_tensor("cvec", [1, L * H], F32, kind="ExternalInput").ap()
    out = nc.dram_tensor("out", [L, rows, H], F32, kind="ExternalOutput").ap()
    # Row r = g*1024 + p*8 + u: partition p owns 8 CONSECUTIVE rows of each
    # group, so a grouped output DMA moves 16 KB contiguous DRAM runs per
    # partition per layer. DMA throughput is packet-rate-bound (~55 ns
    # fixed + ~43 ns/KB per per-partition run on each of 16 engines), so
    # big runs are what buys write bandwidth.
    out_r = out.rearrange("l (g p u) d -> p l g u d", p=P, u=8)

    with tile.TileContext(nc) as tc:
        with (
            tc.tile_pool(name="wpool", bufs=1) as wpool,
            tc.tile_pool(name="xpool", bufs=1) as xpool,
            tc.tile_pool(name="opool", bufs=6) as opool,
            tc.tile_pool(name="psum", bufs=1, space="PSUM") as psum,
        ):
            # ---- input DMAs ----------------------------------------------
            # gft[0] + first x pair first so matmuls start ASAP; everything
            # on the sync ring (gpsimd's TileContext DRAIN is ~17 us and
            # would delay the first matmul). x pairs = 8 KB read packets.
            # x chunks: one per half-group (0.5 MB, 4 KB read packets).
            # gft[0] + the first group's two halves go first so matmuls
            # start ASAP.
            xts = [
                [
                    xpool.tile([P, KC, 512], F16, tag=f"x{g}_{hf}",
                               name=f"x{g}_{hf}")
                    for hf in range(2)
                ]
                for g in range(G)
            ]
            gft_sb = [
                wpool.tile([P, KC, H], F16, tag=f"gf{i}", name=f"gf{i}")
                for i in range(L)
            ]
            # k=0 slices of gft[0] and the first x half land first (~0.25 MB)
            # so the very first matmul issues ~3 us earlier; the DMA feed
            # outruns the matmul stream from then on.
            nc.sync.dma_start(out=gft_sb[0][:, 0:1, :], in_=gft[0][:, 0:1, :])
            nc.sync.dma_start(out=xts[0][0][:, 0:1, :], in_=xt[:, 0, 0, 0:1, :])
            nc.sync.dma_start(out=gft_sb[0][:, 1:, :], in_=gft[0][:, 1:, :])
            nc.sync.dma_start(out=xts[0][0][:, 1:, :], in_=xt[:, 0, 0, 1:, :])
            nc.sync.dma_start(out=xts[0][1], in_=xt[:, 0, 1])
            for i in range(1, L):
                nc.sync.dma_start(out=gft_sb[i], in_=gft[i])
            for g in range(1, G):
                for hf in range(2):
                    nc.sync.dma_start(out=xts[g][hf], in_=xt[:, g, hf])
            cb = wpool.tile([P, L * H], F32, tag="cb")
            c_bcast = bass.AP(
                tensor=cvec.tensor,
                offset=cvec.offset,
                ap=[[0, P]] + list(cvec.ap[1:]),
            )
            nc.scalar.dma_start(out=cb, in_=c_bcast)

            # ---- main loop ------------------------------------------------
            # Per 128-row tile: 16 matmuls accumulate all 4 layers into one
            # [128, 2048] PSUM tile (layer-outer order so each 512-slice
            # finishes early and its bias add overlaps remaining matmuls),
            # then one [128, 2048] sigmoid into the group output buffer.
            # One 4 MB DMA per group of 4 tiles streams the result out.
            # One [128, 512] PSUM tile (= one bank) per (tile, layer) slice,
            # rotating over all 8 banks. PSUM hazards are tile-granular, so
            # a big multi-bank tile would chain each new accumulation to
            # sigmoids of OTHER slices in the same tile (~1.34 us/slice
            # critical cycle); per-bank tiles put the write-after-read
            # partner 8 slices back and Tensor runs free.
            # Layer-outer within each group: the (group, layer) output block
            # [128 rows x 8 tiles x 512] completes every ~7 us, so 2 MB
            # output DMAs (16 KB runs) issue at a smooth cadence. The very
            # last block splits in two so its first half drains while the
            # second half computes.
            for g in range(G):
                for i in range(L):
                    sl = slice(i * H, (i + 1) * H)
                    ob = opool.tile([P, 8, H], F32, tag="ob",
                                    name=f"ob{g}_{i}")
                    last = (g == G - 1 and i == L - 1)
                    for u in range(8):
                        cs = slice((u % 4) * P, (u % 4 + 1) * P)
                        pf = psum.tile([P, H], F32, tag="z", bufs=8,
                                       name=f"z{g}_{u}_{i}")
                        for k in range(KC):
                            nc.tensor.matmul(
                                pf,
                                xts[g][u // 4][:, k, cs],
                                gft_sb[i][:, k, :],
                                start=(k == 0),
                                stop=(k == KC - 1),
                            )
                        nc.vector.tensor_add(pf, pf, cb[:, sl])
                        nc.scalar.activation(ob[:, u, :], pf, SIG)
                        if last and u % 2 == 1:
                            # final block: drain in row pairs (4 KB runs)
                            # right behind each pair of sigmoids so almost
                            # nothing is left after the last ACT
                            q = nc.sync if u % 4 == 1 else nc.scalar
                            q.dma_start(
                                out=out_r[:, i, g, u - 1:u + 1, :],
                                in_=ob[:, u - 1:u + 1, :],
                            )
                    if not last:
                        # alternate hardware rings: two queues keep twice
                        # the packets outstanding, which sets the drain rate
                        # once compute stops feeding new descriptors
                        q = nc.sync if (g * L + i) % 2 == 0 else nc.scalar
                        q.dma_start(out=out_r[:, i, g, :, :], in_=ob)

    nc.compile()
    return nc


def _prep_weights(x_full, Ws, W_ff, b_ff):
    """Host-side: compose G_i, bias rows c_i (fp64), pack for the device."""
    n = x_full.shape[0]
    c = n / (n - 1.0)
    eye = np.eye(H, dtype=np.float64)
    wfT = W_ff.astype(np.float64).T  # [H, OUT]
    total = x_full.sum(axis=0, dtype=np.float64)  # [H]
    # device layout [L, P, KC, H]: partition p, chunk k holds G[k*P+p, :]
    gf = np.empty((L, P, KC, H), dtype=np.float16)
    cv = np.empty((1, L * H), dtype=np.float32)
    M = eye.copy()
    for i in range(L):
        M = M @ (eye + c * Ws[i].astype(np.float64).T)  # M_{i+1}
        Gi = M @ wfT
        gf[i] = Gi.astype(np.float16).reshape(KC, P, H).transpose(1, 0, 2)
        cv[0, i * H:(i + 1) * H] = (
            b_ff.astype(np.float64) + (total / n) @ (wfT - Gi)
        ).astype(np.float32)
    return gf, cv


def _prep_x(x_core):
    """[rows, H] fp32 -> [P, G, 2, KC, 512] fp16.

    Hidden dim on partitions (k*128+p_h); free column uu*128+p_out of
    half hf of group g maps to row g*1024 + p_out*8 + hf*4 + uu, so
    output partitions own 8 consecutive rows (16 KB contiguous output
    DMA runs) while x still loads in 0.5 MB half-group chunks.
    """
    rows = x_core.shape[0]
    g = rows // 1024
    return (
        x_core.reshape(g, P, 2, 4, KC, P)   # [g, p_out, hf, uu, k, p_h]
        .transpose(5, 0, 2, 4, 3, 1)        # [p_h, g, hf, k, uu, p_out]
        .astype(np.float16, order="C")
        .reshape(P, g, 2, KC, 512)
    )


_CACHE = {}


def kernel(input, Ws, W_ff, b_ff):
    x = np.asarray(input, dtype=np.float32)[0]  # [N, H]
    Ws = np.asarray(Ws, dtype=np.float32)
    W_ff = np.asarray(W_ff, dtype=np.float32)
    b_ff = np.asarray(b_ff, dtype=np.float32)
    n, h = x.shape
    rows = n // N_CORES

    if "nc" not in _CACHE:
        _CACHE["nc"] = build(rows=rows)
    nc = _CACHE["nc"]

    gf, cv = _prep_weights(x, Ws, W_ff, b_ff)
    in_maps = [
        {
            "xt": _prep_x(x[c * rows:(c + 1) * rows]),
            "gft": gf,
            "cvec": cv,
        }
        for c in range(N_CORES)
    ]
    res = bass_utils.run_bass_kernel_spmd(
        nc, in_maps, core_ids=list(range(N_CORES)), trace=TRACE
    )
    _CACHE["last_res"] = res
    out = np.concatenate([res.results[c]["out"] for c in range(N_CORES)], axis=1)
    return out.astype(np.float32)
